# revision 21
# baseline (speedup 1.0000x reference)
"""GroupAwareContrastiveLoss Trainium2 kernel (fp8 + fused-DVE version).

Strategy (sharding_hint: shard rows i across 8 cores, replicate codebook):
  - Host normalizes the codebook (zn = z/||z||), scales by SC=64 and
    quantizes to fp8 e4m3. Each core gets a column-rotated copy laid out
    [128, 8, N] so its own 1024 rows land in local columns [0, 1024) --
    the diagonal / range col-blocks are then identical across cores and
    the program stays SPMD while masks remain data-driven.
  - Device computes C = SC^2 * cos via fp8 DoubleRow matmuls (4 per
    128x512 tile, 256-deep contraction each), then ONE fused custom DVE
    op per tile: S = relu(|C| - SC^2*0.1)^2 with a fused row-sum
    accumulator (the full neg/ortho term, scaled by SC^4).
  - Band blocks (in-range cols + diagonal; host-detected signature) get:
    a masked-sum correction (custom TTR vs a host-built in_range|diag
    mask), and the pos chain d2 = sq_i + sq_j - 2*nrm_i*nrm_j*cos ->
    sqrt (ACT) -> fused relu^2*mask reduce (custom DVE).
  - Per-row sums return to host; host scales by 1/SC^4, adds the exact
    j==i ortho constant 0.81, does the O(M) counting/division/mean.
"""

import os
import sys
import numpy as np

if "/opt/trn_rl_repo" not in sys.path:
    sys.path.insert(0, "/opt/trn_rl_repo")

from contextlib import ExitStack
from operator import add as _op_add

import ml_dtypes

import concourse.bass as bass
import concourse.bacc as bacc
import concourse.mybir as mybir
from concourse import tile
from concourse.alu_op_type import AluOpType as ALU
from concourse.bass_utils import run_bass_kernel_spmd

N = 8192          # codebook rows (= cols of the cos matrix)
D = 1024          # feature dim
NCORES = 8
T = 8             # 128-row tiles per core
BLK = 512         # col-block width (one PSUM bank of fp32)
NBLK = N // BLK   # 16
KCH = D // 128    # 8 contraction chunks of 128
KG = KCH // 2     # 4 DoubleRow groups (256-deep each)
ROWS_PER_CORE = T * 128

M_POS = 0.5
M_NEG_SIM = 0.1
LAM_NEG = 1.0
SC = 64.0         # fp8 quantization scale for zn
SC2 = SC * SC
SC4 = SC2 * SC2

FP32 = mybir.dt.float32
BF16 = mybir.dt.bfloat16
FP8 = mybir.dt.float8e4
AF = mybir.ActivationFunctionType

_programs = {}
last_exec_time_ns = None
_last_run = None


# ---------------------------------------------------------------------------
# custom DVE ops (runtime-registered; same mechanism as dve_ops.OPS entries)
# ---------------------------------------------------------------------------
_custom_ops = None


def _get_custom_ops():
    global _custom_ops
    if _custom_ops is not None:
        return _custom_ops

    from concourse import dve_ops
    from concourse.dve_spec import (
        Spec, Src0, Src1, C0, C1, C2, Zero, lower, maxx, relu, sq,
    )
    from concourse.dve_uop import DveOpSpec

    def _sum_ref(body_fn, seed_c0=False):
        def _r(in0, in1, c0, c1, c2):
            b = body_fn(in0, in1, c0, c1, c2).astype(np.float32)
            acc = b.reshape(b.shape[0], -1).sum(axis=-1, keepdims=True)
            return b, (c0 + acc) if seed_c0 else acc
        return _r

    def _dve_relu(x):
        return np.maximum(np.nan_to_num(x, nan=0.0, posinf=np.inf,
                                        neginf=-np.inf), 0)

    specs = [
        # out = relu(|x| - c2)^2 ; accum_out = c0 + row sum (chainable)
        ("TENSOR_NEGABS_RELU_SQ_RED",
         Spec(
             body=sq(relu(maxx(Src0, Zero - Src0) - C2)),
             accum=_op_add,
             accum_init=C0,
             reference=_sum_ref(
                 lambda in0, in1, c0, c1, c2:
                 _dve_relu(np.abs(in0.astype(np.float32)) - c2) ** 2, True),
         )),
        # out = relu(x - c2)^2 * mask ; accum_out = c0 + row sum
        ("TENSOR_POS_RELU_SQ_MASK_RED",
         Spec(
             body=sq(relu(Src0 - C2)) * Src1,
             accum=_op_add,
             accum_init=C0,
             reference=_sum_ref(
                 lambda in0, in1, c0, c1, c2:
                 _dve_relu(in0.astype(np.float32) - c2) ** 2 * in1, True),
         )),
        # out = (x*c0)*y + y^2 + c1   (d2 from scaled cos + nrm_j in one op)
        ("TENSOR_D2_FROM_COS",
         Spec(
             body=(Src0 * C0) * Src1 + sq(Src1) + C1,
             reference=lambda in0, in1, c0, c1, c2: (
                 (in0.astype(np.float32) * c0) * in1 + in1 * in1 + c1
             ).astype(np.float32),
         )),
    ]

    made = []
    for name, spec in specs:
        existing = next((o for o in dve_ops.OPS if o.name == name), None)
        if existing is not None:
            made.append(existing)
            continue
        row = dve_ops._CUSTOM_DVE_ROW_BASE + len(dve_ops.OPS)
        assert row < 0x20, "custom-DVE opcode rows exhausted"
        dve_ops._SUB_OPCODE_FOR_NAME[name] = row
        shas = {}
        from concourse.dve_spec import _has_src1
        rd1 = _has_src1(spec)
        for ver in ("v3", "v4"):
            u = lower(spec, ver=ver)
            shas[ver] = DveOpSpec(name=name, opcode=row, uops=u,
                                  rd1_en=rd1).sha(ver)
        op = dve_ops.DveOp(name, spec, subdim=False, uops_sha=shas)
        dve_ops.OPS.append(op)
        dve_ops.CUSTOM_DVE_SPECS[name] = spec
        made.append(op)

    _custom_ops = tuple(made)
    return _custom_ops


# ---------------------------------------------------------------------------
# program builder
# ---------------------------------------------------------------------------
def _build_program(corr_sig, act_sig):
    """corr_sig/act_sig: tuple over t of sorted tuple of col-blocks that get
    the neg-correction / pos-chain ops (union across cores)."""
    NEG_OP, POS_OP, D2_OP = _get_custom_ops()

    nc = bacc.Bacc("TRN2", target_bir_lowering=False, debug=False,
                   num_devices=1)

    n_corr = sum(len(c) for c in corr_sig)
    n_act = sum(len(a) for a in act_sig)

    zn8 = nc.declare_dram_parameter("zn8", [128, KCH, N], FP8, isOutput=False)
    masks = nc.declare_dram_parameter(
        "masks", [max(n_corr, 1), 128, 2, BLK], BF16, isOutput=False)
    bc = nc.declare_dram_parameter(
        "bc", [max(n_act, 1), 128, BLK], FP32, isOutput=False)
    scal = nc.declare_dram_parameter("scal", [T, 128, 2], FP32, isOutput=False)
    sums = nc.declare_dram_parameter("sums", [128, 3 * T], FP32, isOutput=True)

    dma = nc.sync.dma_start      # rhs stream + outputs
    dma_aux = nc.gpsimd.dma_start  # resident loads, off the critical queue

    corr_idx = {}
    for t in range(T):
        for b in corr_sig[t]:
            corr_idx[(t, b)] = len(corr_idx)
    act_idx = {}
    for t in range(T):
        for b in act_sig[t]:
            act_idx[(t, b)] = len(act_idx)

    with tile.TileContext(nc) as tc, ExitStack() as ctx:
        res_pool = ctx.enter_context(tc.tile_pool(name="res", bufs=1))
        rhs_pool = ctx.enter_context(tc.tile_pool(name="rhs", bufs=2))
        psum_pool = ctx.enter_context(
            tc.tile_pool(name="psum", bufs=8, space="PSUM"))
        s_pool = ctx.enter_context(tc.tile_pool(name="spool", bufs=3))
        band_pool = ctx.enter_context(tc.tile_pool(name="band", bufs=2))

        # lhs on the fast sync queue, chunked per row-tile so the first
        # matmul group only waits for its own 128 columns; other resident
        # loads go to the gpsimd queue.
        lhs = res_pool.tile([128, KCH, ROWS_PER_CORE], FP8, tag="lhs",
                            name="lhs")
        dma(lhs[:, :, 0:128], zn8[:, :, 0:128])

        scal_sb = []
        for t in range(T):
            st = res_pool.tile([128, 2], FP32, tag=f"scal{t}", name=f"scal{t}")
            dma_aux(st[:], scal[t])
            scal_sb.append(st)

        mask_sb = {}
        for (t, b), idx in corr_idx.items():
            mt = res_pool.tile([128, 2, BLK], BF16, tag=f"mk{idx}",
                               name=f"mk{idx}")
            dma_aux(mt[:], masks[idx])
            mask_sb[(t, b)] = mt
        bc_sb = {}
        for (t, b), idx in act_idx.items():
            bt = res_pool.tile([128, BLK], FP32, tag=f"bc{idx}",
                               name=f"bc{idx}")
            dma_aux(bt[:], bc[idx])
            bc_sb[(t, b)] = bt

        # one accumulator tile: cols [3t, 3t+1, 3t+2] = pos, negfull, negcorr
        acc = res_pool.tile([128, 3 * T], FP32, tag="acc", name="acc")

    # main loop
        neg_seen = [False] * T
        corr_seen = [False] * T
        pos_seen = [False] * T
        m_neg_dev = float(M_NEG_SIM * SC2)

        for b in range(NBLK):
            rhs = rhs_pool.tile([128, KCH, BLK], FP8, tag="rhs", name="rhs")
            dma(rhs[:], zn8[:, :, b * BLK:(b + 1) * BLK])
            if b == 0:
                # rest of lhs streams in behind rhs0
                dma(lhs[:, :, 128:ROWS_PER_CORE], zn8[:, :, 128:ROWS_PER_CORE])

            for t in range(T):
                C = psum_pool.tile([128, BLK], FP32, tag="C", name="C")
                for g in range(KG):
                    nc.tensor.matmul(
                        C[:],
                        lhs[:, 2 * g:2 * g + 2, t * 128:(t + 1) * 128],
                        rhs[:, 2 * g:2 * g + 2, :],
                        start=(g == 0),
                        stop=(g == KG - 1),
                        perf_mode=mybir.MatmulPerfMode.DoubleRow,
                    )

                S = s_pool.tile([128, BLK], BF16, tag="S", name="S")
                nfc = acc[:, 3 * t + 1:3 * t + 2]
                nc.vector._custom_dve(
                    NEG_OP, out=S[:], in0=C[:], imm2=m_neg_dev,
                    s0=(nfc if neg_seen[t] else 0.0),
                    accum_out=nfc,
                )
                neg_seen[t] = True

                if (t, b) in corr_idx:
                    mt = mask_sb[(t, b)]
                    junk = s_pool.tile([128, BLK], BF16, tag="junk",
                                       name="junk")
                    from concourse.dve_ops import TENSOR_TENSOR_REDUCE
                    ncc = acc[:, 3 * t + 2:3 * t + 3]
                    nc.vector._custom_dve(
                        TENSOR_TENSOR_REDUCE, out=junk[:], in0=S[:],
                        in1=mt[:, 0, :],
                        s0=(ncc if corr_seen[t] else 0.0), s1=1.0,
                        accum_out=ncc,
                    )
                    corr_seen[t] = True

                if (t, b) in act_idx:
                    bt = bc_sb[(t, b)]
                    st = scal_sb[t]
                    w = band_pool.tile([128, BLK], FP32, tag="w", name="w")
                    nc.vector._custom_dve(
                        D2_OP, out=w[:], in0=C[:], in1=bt[:],
                        s0=st[:, 0:1], s1=st[:, 1:2],
                    )
                    Dt = band_pool.tile([128, BLK], BF16, tag="Dt", name="Dt")
                    nc.scalar.activation(Dt[:], w[:], AF.Sqrt)
                    junk2 = s_pool.tile([128, BLK], BF16, tag="junk2",
                                        name="junk2")
                    mt = mask_sb[(t, b)]
                    pc = acc[:, 3 * t:3 * t + 1]
                    nc.vector._custom_dve(
                        POS_OP, out=junk2[:], in0=Dt[:], in1=mt[:, 1, :],
                        imm2=float(M_POS),
                        s0=(pc if pos_seen[t] else 0.0),
                        accum_out=pc,
                    )
                    pos_seen[t] = True

        # zero any never-written accumulator columns, then one output DMA
        for t in range(T):
            if not pos_seen[t]:
                nc.vector.memset(acc[:, 3 * t:3 * t + 1], 0.0)
            if not corr_seen[t]:
                nc.vector.memset(acc[:, 3 * t + 2:3 * t + 3], 0.0)
        dma(sums[:], acc[:])

    nc.compile()
    return nc


# ---------------------------------------------------------------------------
# host-side input prep
# ---------------------------------------------------------------------------
def _prepare_inputs(codebook, starts, ends):
    cb = np.asarray(codebook, dtype=np.float32)
    s_arr = np.asarray(starts).astype(np.int64)
    e_arr = np.asarray(ends).astype(np.int64)

    sq64 = np.sum(cb.astype(np.float64) ** 2, axis=-1)
    nrm = np.sqrt(sq64).astype(np.float32)
    sq = sq64.astype(np.float32)
    zn = cb / nrm[:, None]
    zn8 = (zn * SC).astype(ml_dtypes.float8_e4m3)  # [N, D]

    s_cl = np.maximum(s_arr, 0)
    e_cl = np.minimum(e_arr, N - 1)
    nonempty = s_cl <= e_cl

    # ---- SPMD signature: union of needed blocks across cores ----
    corr_sig = [set() for _ in range(T)]
    act_sig = [set() for _ in range(T)]
    per_core = []
    for c in range(NCORES):
        off = c * ROWS_PER_CORE
        r = np.arange(ROWS_PER_CORE)
        gi = off + r
        sL = (s_cl[gi] - off) % N
        eL = (e_cl[gi] - off) % N
        wrap = nonempty[gi] & (sL > eL)
        ne = nonempty[gi]
        # interval list per row in local coords
        i1s = np.where(ne, np.where(wrap, 0, sL), 1)
        i1e = np.where(ne, eL, 0)
        i1v = ne.copy()
        i2s = np.where(wrap, sL, 1)
        i2e = np.where(wrap, np.int64(N - 1), 0)
        i2v = wrap.copy()
        per_core.append((off, i1s, i1e, i1v, i2s, i2e, i2v))
        for t in range(T):
            rt = slice(t * 128, (t + 1) * 128)
            for ss, ee, vv in ((i1s[rt], i1e[rt], i1v[rt]),
                               (i2s[rt], i2e[rt], i2v[rt])):
                ok = vv & (ss <= ee)
                if not ok.any():
                    continue
                for lo, hi in zip(ss[ok] // BLK, ee[ok] // BLK):
                    for bb in range(int(lo), int(hi) + 1):
                        act_sig[t].add(bb)
                        corr_sig[t].add(bb)
            corr_sig[t].add(t // 4)  # diagonal block always corrected

    corr_sig = tuple(tuple(sorted(s)) for s in corr_sig)
    act_sig = tuple(tuple(sorted(s)) for s in act_sig)

    corr_list = [(t, b) for t in range(T) for b in corr_sig[t]]
    act_list = [(t, b) for t in range(T) for b in act_sig[t]]
    n_corr, n_act = len(corr_list), len(act_list)

    # ---- per-core input maps ----
    in_maps = []
    for c in range(NCORES):
        off, i1s, i1e, i1v, i2s, i2e, i2v = per_core[c]
        # rotated fp8 matrix, layout [128, KCH, N]
        rolled = np.roll(zn8, -off, axis=0)              # [N, D]
        zn8_c = np.ascontiguousarray(
            rolled.T.reshape(KCH, 128, N).transpose(1, 0, 2))

        r = np.arange(ROWS_PER_CORE)
        gi = off + r

        scal_c = np.zeros((T, 128, 2), dtype=np.float32)
        flat = scal_c.reshape(ROWS_PER_CORE, 2)
        flat[:, 0] = -2.0 * nrm[gi] / SC2
        flat[:, 1] = sq[gi]

        # masks per corr entry: [n_corr, 128, 2, BLK] (mcorr, mpos)
        masks_c = np.zeros((max(n_corr, 1), 128, 2, BLK), dtype=np.float32)
        for idx, (t, b) in enumerate(corr_list):
            rt = np.arange(t * 128, (t + 1) * 128)
            cols = np.arange(b * BLK, (b + 1) * BLK)[None, :]
            inr = np.zeros((128, BLK), dtype=bool)
            for ss, ee, vv in ((i1s[rt], i1e[rt], i1v[rt]),
                               (i2s[rt], i2e[rt], i2v[rt])):
                inr |= vv[:, None] & (cols >= ss[:, None]) & \
                       (cols <= ee[:, None])
            diag = cols == rt[:, None]
            masks_c[idx, :, 0, :] = (inr | diag).astype(np.float32)
            masks_c[idx, :, 1, :] = (inr & ~diag).astype(np.float32)
        masks_c = masks_c.astype(ml_dtypes.bfloat16)

        # bc per act entry: [n_act, 128, BLK] (nrm_j broadcast; sq_j = nrm^2
        # is recomputed on-device inside the fused d2 op)
        nrm_rot = np.roll(nrm, -off)
        bc_c = np.zeros((max(n_act, 1), 128, BLK), dtype=np.float32)
        for idx, (t, b) in enumerate(act_list):
            bc_c[idx] = nrm_rot[b * BLK:(b + 1) * BLK][None, :]

        in_maps.append({
            "zn8": zn8_c, "masks": masks_c, "bc": bc_c, "scal": scal_c,
        })

    return in_maps, corr_sig, act_sig


# ---------------------------------------------------------------------------
# host-side finalize
# ---------------------------------------------------------------------------
def _host_finalize(pos_dev, neg_dev, starts, ends, M):
    s_arr = np.asarray(starts).astype(np.int64)[:M]
    e_arr = np.asarray(ends).astype(np.int64)[:M]
    i_arr = np.arange(M, dtype=np.int64)

    lo = np.maximum(s_arr, 0)
    hi = np.minimum(e_arr, N - 1)
    cnt_in = np.maximum(0, hi - lo + 1)
    in_i = ((i_arr >= s_arr) & (i_arr <= e_arr)).astype(np.int64)
    pos_cnt = cnt_in - in_i
    neg_cnt = N - cnt_in + in_i

    diag_term = (1.0 - M_NEG_SIM) ** 2  # exact j==i ortho entry
    pos_sum = pos_dev[:M].astype(np.float64)
    neg_sum = neg_dev[:M].astype(np.float64) + diag_term

    pos_pull = pos_sum / np.maximum(pos_cnt, 1)
    ortho = neg_sum / np.maximum(neg_cnt, 1)
    valid = (pos_cnt > 0) & (neg_cnt > 0)
    per_row = np.where(valid, pos_pull + LAM_NEG * ortho, 0.0)
    cnt = int(valid.sum())
    if cnt > 0:
        return np.float32(per_row.sum() / cnt)
    return np.float32(0.0)


# ---------------------------------------------------------------------------
# NTFF trace hook (profiling only; inert when KTRACE is unset)
# ---------------------------------------------------------------------------
def _install_trace_hook():
    import types
    try:
        import antenv
        if "antenv.axon_hooks" not in sys.modules:
            mod = types.ModuleType("antenv.axon_hooks")
            state = {"hook": None}
            mod.set_axon_ntff_profile_hook = \
                lambda h: state.__setitem__("hook", h)
            mod.get_axon_ntff_profile_hook = lambda: state["hook"]
            sys.modules["antenv.axon_hooks"] = mod
            antenv.axon_hooks = mod
        from antenv.axon_hooks import (
            get_axon_ntff_profile_hook, set_axon_ntff_profile_hook,
        )
        if get_axon_ntff_profile_hook() is None:
            from trn_agent_boot.trn_boot import _ntff_profile_via_ctypes
            set_axon_ntff_profile_hook(
                _ntff_profile_via_ctypes("/opt/axon/libaxon_pjrt.so"))
        import concourse.bass_utils as bu
        if not getattr(bu.upload_artifacts, "_stubbed", False):
            def _noop_upload(tmpdir):
                return tmpdir
            _noop_upload._stubbed = True
            bu.upload_artifacts = _noop_upload
        return True
    except Exception:
        return False


# ---------------------------------------------------------------------------
# entry point
# ---------------------------------------------------------------------------
def kernel(codebook, starts, ends, max_i):
    global last_exec_time_ns, _last_run

    codebook = np.asarray(codebook)
    assert codebook.shape == (N, D), codebook.shape
    M = min(N, int(max_i) + 1)

    in_maps, corr_sig, act_sig = _prepare_inputs(codebook, starts, ends)

    key = (corr_sig, act_sig)
    if key not in _programs:
        _programs[key] = _build_program(corr_sig, act_sig)
    nc = _programs[key]

    trace = bool(os.environ.get("KTRACE"))
    if trace:
        trace = _install_trace_hook()
    res = run_bass_kernel_spmd(
        nc, in_maps, core_ids=list(range(NCORES)), trace=trace)
    last_exec_time_ns = res.exec_time_ns
    _last_run = res

    pos_dev = np.empty(N, dtype=np.float64)
    neg_dev = np.empty(N, dtype=np.float64)
    for c in range(NCORES):
        s = res.results[c]["sums"].astype(np.float64)  # (128, 3*T)
        s = s.reshape(128, T, 3).transpose(1, 0, 2)    # (T, 128, 3)
        off = c * ROWS_PER_CORE
        pos_dev[off:off + ROWS_PER_CORE] = s[..., 0].reshape(-1)
        neg_dev[off:off + ROWS_PER_CORE] = \
            (s[..., 1] - s[..., 2]).reshape(-1) / SC4

    return np.asarray(_host_finalize(pos_dev, neg_dev, starts, ends, M))


# revision 24
# speedup vs baseline: 1.1818x; 1.1818x over previous
"""GroupAwareContrastiveLoss Trainium2 kernel (fp8 + fused-DVE version).

Strategy (sharding_hint: shard rows i across 8 cores, replicate codebook):
  - Host normalizes the codebook (zn = z/||z||), scales by SC=64 and
    quantizes to fp8 e4m3. Each core gets a column-rotated copy laid out
    [128, 8, N] so its own 1024 rows land in local columns [0, 1024) --
    the diagonal / range col-blocks are then identical across cores and
    the program stays SPMD while masks remain data-driven.
  - Device computes C = SC^2 * cos via fp8 DoubleRow matmuls (4 per
    128x512 tile, 256-deep contraction each), then ONE fused custom DVE
    op per tile: S = relu(|C| - SC^2*0.1)^2 with a fused row-sum
    accumulator (the full neg/ortho term, scaled by SC^4).
  - Band blocks (in-range cols + diagonal; host-detected signature) get:
    a masked-sum correction (custom TTR vs a host-built in_range|diag
    mask), and the pos chain d2 = sq_i + sq_j - 2*nrm_i*nrm_j*cos ->
    sqrt (ACT) -> fused relu^2*mask reduce (custom DVE).
  - Per-row sums return to host; host scales by 1/SC^4, adds the exact
    j==i ortho constant 0.81, does the O(M) counting/division/mean.
"""

import os
import sys
import numpy as np

if "/opt/trn_rl_repo" not in sys.path:
    sys.path.insert(0, "/opt/trn_rl_repo")

from contextlib import ExitStack
from operator import add as _op_add

import ml_dtypes

import concourse.bass as bass
import concourse.bacc as bacc
import concourse.mybir as mybir
from concourse import tile
from concourse.alu_op_type import AluOpType as ALU
from concourse.bass_utils import run_bass_kernel_spmd

N = 8192          # codebook rows (= cols of the cos matrix)
D = 1024          # feature dim
NCORES = 8
T = 8             # 128-row tiles per core
BLK = 512         # col-block width (one PSUM bank of fp32)
NBLK = N // BLK   # 16
KCH = D // 128    # 8 contraction chunks of 128
KG = KCH // 2     # 4 DoubleRow groups (256-deep each)
ROWS_PER_CORE = T * 128

M_POS = 0.5
M_NEG_SIM = 0.1
LAM_NEG = 1.0
SC = 64.0         # fp8 quantization scale for zn
SC2 = SC * SC
SC4 = SC2 * SC2

FP32 = mybir.dt.float32
BF16 = mybir.dt.bfloat16
FP8 = mybir.dt.float8e4
AF = mybir.ActivationFunctionType

_programs = {}
last_exec_time_ns = None
_last_run = None


# ---------------------------------------------------------------------------
# custom DVE ops (runtime-registered; same mechanism as dve_ops.OPS entries)
# ---------------------------------------------------------------------------
_custom_ops = None


def _get_custom_ops():
    global _custom_ops
    if _custom_ops is not None:
        return _custom_ops

    from concourse import dve_ops
    from concourse.dve_spec import (
        Spec, Src0, Src1, C0, C1, C2, Zero, lower, maxx, relu, sq,
    )
    from concourse.dve_uop import DveOpSpec

    def _sum_ref(body_fn, seed_c0=False):
        def _r(in0, in1, c0, c1, c2):
            b = body_fn(in0, in1, c0, c1, c2).astype(np.float32)
            acc = b.reshape(b.shape[0], -1).sum(axis=-1, keepdims=True)
            return b, (c0 + acc) if seed_c0 else acc
        return _r

    def _dve_relu(x):
        return np.maximum(np.nan_to_num(x, nan=0.0, posinf=np.inf,
                                        neginf=-np.inf), 0)

    specs = [
        # out = relu(|x| - c2)^2 ; accum_out = c0 + row sum (chainable)
        ("TENSOR_NEGABS_RELU_SQ_RED",
         Spec(
             body=sq(relu(maxx(Src0, Zero - Src0) - C2)),
             accum=_op_add,
             accum_init=C0,
             reference=_sum_ref(
                 lambda in0, in1, c0, c1, c2:
                 _dve_relu(np.abs(in0.astype(np.float32)) - c2) ** 2, True),
         )),
        # out = relu(x - c2)^2 * mask ; accum_out = c0 + row sum
        ("TENSOR_POS_RELU_SQ_MASK_RED",
         Spec(
             body=sq(relu(Src0 - C2)) * Src1,
             accum=_op_add,
             accum_init=C0,
             reference=_sum_ref(
                 lambda in0, in1, c0, c1, c2:
                 _dve_relu(in0.astype(np.float32) - c2) ** 2 * in1, True),
         )),
        # out = (x*c0)*y + y^2 + c1   (d2 from scaled cos + nrm_j in one op)
        ("TENSOR_D2_FROM_COS",
         Spec(
             body=(Src0 * C0) * Src1 + sq(Src1) + C1,
             reference=lambda in0, in1, c0, c1, c2: (
                 (in0.astype(np.float32) * c0) * in1 + in1 * in1 + c1
             ).astype(np.float32),
         )),
    ]

    made = []
    for name, spec in specs:
        existing = next((o for o in dve_ops.OPS if o.name == name), None)
        if existing is not None:
            made.append(existing)
            continue
        row = dve_ops._CUSTOM_DVE_ROW_BASE + len(dve_ops.OPS)
        assert row < 0x20, "custom-DVE opcode rows exhausted"
        dve_ops._SUB_OPCODE_FOR_NAME[name] = row
        shas = {}
        from concourse.dve_spec import _has_src1
        rd1 = _has_src1(spec)
        for ver in ("v3", "v4"):
            u = lower(spec, ver=ver)
            shas[ver] = DveOpSpec(name=name, opcode=row, uops=u,
                                  rd1_en=rd1).sha(ver)
        op = dve_ops.DveOp(name, spec, subdim=False, uops_sha=shas)
        dve_ops.OPS.append(op)
        dve_ops.CUSTOM_DVE_SPECS[name] = spec
        made.append(op)

    _custom_ops = tuple(made)
    return _custom_ops


# ---------------------------------------------------------------------------
# program builder
# ---------------------------------------------------------------------------
def _build_program(corr_sig, act_sig):
    """corr_sig/act_sig: tuple over t of sorted tuple of col-blocks that get
    the neg-correction / pos-chain ops (union across cores)."""
    NEG_OP, POS_OP, D2_OP = _get_custom_ops()

    nc = bacc.Bacc("TRN2", target_bir_lowering=False, debug=False,
                   num_devices=1)

    n_corr = sum(len(c) for c in corr_sig)
    n_act = sum(len(a) for a in act_sig)

    zn8 = nc.declare_dram_parameter("zn8", [128, KCH, N], FP8, isOutput=False)
    masks = nc.declare_dram_parameter(
        "masks", [max(n_corr, 1), 128, 2, BLK], BF16, isOutput=False)
    bc = nc.declare_dram_parameter(
        "bc", [max(n_act, 1), 128, BLK], FP32, isOutput=False)
    scal = nc.declare_dram_parameter("scal", [T, 128, 2], FP32, isOutput=False)
    sums = nc.declare_dram_parameter("sums", [128, 3 * T], FP32, isOutput=True)

    dma = nc.sync.dma_start      # rhs stream + outputs
    dma_aux = nc.gpsimd.dma_start  # resident loads, off the critical queue

    corr_idx = {}
    for t in range(T):
        for b in corr_sig[t]:
            corr_idx[(t, b)] = len(corr_idx)
    act_idx = {}
    for t in range(T):
        for b in act_sig[t]:
            act_idx[(t, b)] = len(act_idx)

    with tile.TileContext(nc) as tc, ExitStack() as ctx:
        res_pool = ctx.enter_context(tc.tile_pool(name="res", bufs=1))
        rhs_pool = ctx.enter_context(tc.tile_pool(name="rhs", bufs=2))
        psum_pool = ctx.enter_context(
            tc.tile_pool(name="psum", bufs=8, space="PSUM"))
        s_pool = ctx.enter_context(tc.tile_pool(name="spool", bufs=3))
        band_pool = ctx.enter_context(tc.tile_pool(name="band", bufs=2))

        # lhs on the fast sync queue, chunked per row-tile so the first
        # matmul group only waits for its own 128 columns; other resident
        # loads go to the gpsimd queue.
        lhs = res_pool.tile([128, KCH, ROWS_PER_CORE], FP8, tag="lhs",
                            name="lhs")
        dma(lhs[:, :, 0:128], zn8[:, :, 0:128])

        scal_sb = []
        for t in range(T):
            st = res_pool.tile([128, 2], FP32, tag=f"scal{t}", name=f"scal{t}")
            dma_aux(st[:], scal[t])
            scal_sb.append(st)

        mask_sb = {}
        for (t, b), idx in corr_idx.items():
            mt = res_pool.tile([128, 2, BLK], BF16, tag=f"mk{idx}",
                               name=f"mk{idx}")
            dma_aux(mt[:], masks[idx])
            mask_sb[(t, b)] = mt
        bc_sb = {}
        for (t, b), idx in act_idx.items():
            bt = res_pool.tile([128, BLK], FP32, tag=f"bc{idx}",
                               name=f"bc{idx}")
            dma_aux(bt[:], bc[idx])
            bc_sb[(t, b)] = bt

        negfull = [res_pool.tile([128, NBLK], FP32, tag=f"nf{t}", name=f"nf{t}")
                   for t in range(T)]
        negcorr = [res_pool.tile([128, max(len(corr_sig[t]), 1)], FP32,
                                 tag=f"ncr{t}", name=f"ncr{t}")
                   for t in range(T)]
        posacc = [res_pool.tile([128, max(len(act_sig[t]), 1)], FP32,
                                tag=f"pa{t}", name=f"pa{t}")
                  for t in range(T)]
        acc = res_pool.tile([128, 3 * T], FP32, tag="acc", name="acc")

    # main loop
        ncorr_col = [0] * T
        pos_col = [0] * T
        m_neg_dev = float(M_NEG_SIM * SC2)

        for b in range(NBLK):
            rhs = rhs_pool.tile([128, KCH, BLK], FP8, tag="rhs", name="rhs")
            dma(rhs[:], zn8[:, :, b * BLK:(b + 1) * BLK])
            if b == 0:
                # rest of lhs streams in behind rhs0
                dma(lhs[:, :, 128:ROWS_PER_CORE], zn8[:, :, 128:ROWS_PER_CORE])

            for t in range(T):
                C = psum_pool.tile([128, BLK], FP32, tag="C", name="C")
                for g in range(KG):
                    nc.tensor.matmul(
                        C[:],
                        lhs[:, 2 * g:2 * g + 2, t * 128:(t + 1) * 128],
                        rhs[:, 2 * g:2 * g + 2, :],
                        start=(g == 0),
                        stop=(g == KG - 1),
                        perf_mode=mybir.MatmulPerfMode.DoubleRow,
                    )

                S = s_pool.tile([128, BLK], BF16, tag="S", name="S")
                nc.vector._custom_dve(
                    NEG_OP, out=S[:], in0=C[:], imm2=m_neg_dev,
                    accum_out=negfull[t][:, b:b + 1],
                )

                if (t, b) in corr_idx:
                    mt = mask_sb[(t, b)]
                    junk = s_pool.tile([128, BLK], BF16, tag="junk",
                                       name="junk")
                    from concourse.dve_ops import TENSOR_TENSOR_REDUCE
                    nc.vector._custom_dve(
                        TENSOR_TENSOR_REDUCE, out=junk[:], in0=S[:],
                        in1=mt[:, 0, :], s0=0.0, s1=1.0,
                        accum_out=negcorr[t][:, ncorr_col[t]:ncorr_col[t] + 1],
                    )
                    ncorr_col[t] += 1

                if (t, b) in act_idx:
                    bt = bc_sb[(t, b)]
                    st = scal_sb[t]
                    w = band_pool.tile([128, BLK], FP32, tag="w", name="w")
                    nc.vector._custom_dve(
                        D2_OP, out=w[:], in0=C[:], in1=bt[:],
                        s0=st[:, 0:1], s1=st[:, 1:2],
                    )
                    Dt = band_pool.tile([128, BLK], BF16, tag="Dt", name="Dt")
                    nc.scalar.activation(Dt[:], w[:], AF.Sqrt)
                    junk2 = s_pool.tile([128, BLK], BF16, tag="junk2",
                                        name="junk2")
                    mt = mask_sb[(t, b)]
                    nc.vector._custom_dve(
                        POS_OP, out=junk2[:], in0=Dt[:], in1=mt[:, 1, :],
                        imm2=float(M_POS), s0=0.0,
                        accum_out=posacc[t][:, pos_col[t]:pos_col[t] + 1],
                    )
                    pos_col[t] += 1

        # finalize: reduce into the combined tile, one output DMA
        for t in range(T):
            if pos_col[t] > 0:
                nc.vector.tensor_reduce(
                    acc[:, 3 * t:3 * t + 1], posacc[t][:, 0:pos_col[t]],
                    axis=mybir.AxisListType.X, op=ALU.add)
            else:
                nc.vector.memset(acc[:, 3 * t:3 * t + 1], 0.0)
            nc.vector.tensor_reduce(
                acc[:, 3 * t + 1:3 * t + 2], negfull[t][:],
                axis=mybir.AxisListType.X, op=ALU.add)
            if ncorr_col[t] > 0:
                nc.vector.tensor_reduce(
                    acc[:, 3 * t + 2:3 * t + 3], negcorr[t][:, 0:ncorr_col[t]],
                    axis=mybir.AxisListType.X, op=ALU.add)
            else:
                nc.vector.memset(acc[:, 3 * t + 2:3 * t + 3], 0.0)
        dma(sums[:], acc[:])

    nc.compile()
    return nc


# ---------------------------------------------------------------------------
# host-side input prep
# ---------------------------------------------------------------------------
def _prepare_inputs(codebook, starts, ends):
    cb = np.asarray(codebook, dtype=np.float32)
    s_arr = np.asarray(starts).astype(np.int64)
    e_arr = np.asarray(ends).astype(np.int64)

    sq64 = np.sum(cb.astype(np.float64) ** 2, axis=-1)
    nrm = np.sqrt(sq64).astype(np.float32)
    sq = sq64.astype(np.float32)
    zn = cb / nrm[:, None]
    zn8 = (zn * SC).astype(ml_dtypes.float8_e4m3)  # [N, D]

    s_cl = np.maximum(s_arr, 0)
    e_cl = np.minimum(e_arr, N - 1)
    nonempty = s_cl <= e_cl

    # ---- SPMD signature: union of needed blocks across cores ----
    corr_sig = [set() for _ in range(T)]
    act_sig = [set() for _ in range(T)]
    per_core = []
    for c in range(NCORES):
        off = c * ROWS_PER_CORE
        r = np.arange(ROWS_PER_CORE)
        gi = off + r
        sL = (s_cl[gi] - off) % N
        eL = (e_cl[gi] - off) % N
        wrap = nonempty[gi] & (sL > eL)
        ne = nonempty[gi]
        # interval list per row in local coords
        i1s = np.where(ne, np.where(wrap, 0, sL), 1)
        i1e = np.where(ne, eL, 0)
        i1v = ne.copy()
        i2s = np.where(wrap, sL, 1)
        i2e = np.where(wrap, np.int64(N - 1), 0)
        i2v = wrap.copy()
        per_core.append((off, i1s, i1e, i1v, i2s, i2e, i2v))
        for t in range(T):
            rt = slice(t * 128, (t + 1) * 128)
            for ss, ee, vv in ((i1s[rt], i1e[rt], i1v[rt]),
                               (i2s[rt], i2e[rt], i2v[rt])):
                ok = vv & (ss <= ee)
                if not ok.any():
                    continue
                for lo, hi in zip(ss[ok] // BLK, ee[ok] // BLK):
                    for bb in range(int(lo), int(hi) + 1):
                        act_sig[t].add(bb)
                        corr_sig[t].add(bb)
            corr_sig[t].add(t // 4)  # diagonal block always corrected

    corr_sig = tuple(tuple(sorted(s)) for s in corr_sig)
    act_sig = tuple(tuple(sorted(s)) for s in act_sig)

    corr_list = [(t, b) for t in range(T) for b in corr_sig[t]]
    act_list = [(t, b) for t in range(T) for b in act_sig[t]]
    n_corr, n_act = len(corr_list), len(act_list)

    # ---- per-core input maps ----
    in_maps = []
    for c in range(NCORES):
        off, i1s, i1e, i1v, i2s, i2e, i2v = per_core[c]
        # rotated fp8 matrix, layout [128, KCH, N]
        rolled = np.roll(zn8, -off, axis=0)              # [N, D]
        zn8_c = np.ascontiguousarray(
            rolled.T.reshape(KCH, 128, N).transpose(1, 0, 2))

        r = np.arange(ROWS_PER_CORE)
        gi = off + r

        scal_c = np.zeros((T, 128, 2), dtype=np.float32)
        flat = scal_c.reshape(ROWS_PER_CORE, 2)
        flat[:, 0] = -2.0 * nrm[gi] / SC2
        flat[:, 1] = sq[gi]

        # masks per corr entry: [n_corr, 128, 2, BLK] (mcorr, mpos)
        masks_c = np.zeros((max(n_corr, 1), 128, 2, BLK), dtype=np.float32)
        for idx, (t, b) in enumerate(corr_list):
            rt = np.arange(t * 128, (t + 1) * 128)
            cols = np.arange(b * BLK, (b + 1) * BLK)[None, :]
            inr = np.zeros((128, BLK), dtype=bool)
            for ss, ee, vv in ((i1s[rt], i1e[rt], i1v[rt]),
                               (i2s[rt], i2e[rt], i2v[rt])):
                inr |= vv[:, None] & (cols >= ss[:, None]) & \
                       (cols <= ee[:, None])
            diag = cols == rt[:, None]
            masks_c[idx, :, 0, :] = (inr | diag).astype(np.float32)
            masks_c[idx, :, 1, :] = (inr & ~diag).astype(np.float32)
        masks_c = masks_c.astype(ml_dtypes.bfloat16)

        # bc per act entry: [n_act, 128, BLK] (nrm_j broadcast; sq_j = nrm^2
        # is recomputed on-device inside the fused d2 op)
        nrm_rot = np.roll(nrm, -off)
        bc_c = np.zeros((max(n_act, 1), 128, BLK), dtype=np.float32)
        for idx, (t, b) in enumerate(act_list):
            bc_c[idx] = nrm_rot[b * BLK:(b + 1) * BLK][None, :]

        in_maps.append({
            "zn8": zn8_c, "masks": masks_c, "bc": bc_c, "scal": scal_c,
        })

    return in_maps, corr_sig, act_sig


# ---------------------------------------------------------------------------
# host-side finalize
# ---------------------------------------------------------------------------
def _host_finalize(pos_dev, neg_dev, starts, ends, M):
    s_arr = np.asarray(starts).astype(np.int64)[:M]
    e_arr = np.asarray(ends).astype(np.int64)[:M]
    i_arr = np.arange(M, dtype=np.int64)

    lo = np.maximum(s_arr, 0)
    hi = np.minimum(e_arr, N - 1)
    cnt_in = np.maximum(0, hi - lo + 1)
    in_i = ((i_arr >= s_arr) & (i_arr <= e_arr)).astype(np.int64)
    pos_cnt = cnt_in - in_i
    neg_cnt = N - cnt_in + in_i

    diag_term = (1.0 - M_NEG_SIM) ** 2  # exact j==i ortho entry
    pos_sum = pos_dev[:M].astype(np.float64)
    neg_sum = neg_dev[:M].astype(np.float64) + diag_term

    pos_pull = pos_sum / np.maximum(pos_cnt, 1)
    ortho = neg_sum / np.maximum(neg_cnt, 1)
    valid = (pos_cnt > 0) & (neg_cnt > 0)
    per_row = np.where(valid, pos_pull + LAM_NEG * ortho, 0.0)
    cnt = int(valid.sum())
    if cnt > 0:
        return np.float32(per_row.sum() / cnt)
    return np.float32(0.0)


# ---------------------------------------------------------------------------
# NTFF trace hook (profiling only; inert when KTRACE is unset)
# ---------------------------------------------------------------------------
def _install_trace_hook():
    import types
    try:
        import antenv
        if "antenv.axon_hooks" not in sys.modules:
            mod = types.ModuleType("antenv.axon_hooks")
            state = {"hook": None}
            mod.set_axon_ntff_profile_hook = \
                lambda h: state.__setitem__("hook", h)
            mod.get_axon_ntff_profile_hook = lambda: state["hook"]
            sys.modules["antenv.axon_hooks"] = mod
            antenv.axon_hooks = mod
        from antenv.axon_hooks import (
            get_axon_ntff_profile_hook, set_axon_ntff_profile_hook,
        )
        if get_axon_ntff_profile_hook() is None:
            from trn_agent_boot.trn_boot import _ntff_profile_via_ctypes
            set_axon_ntff_profile_hook(
                _ntff_profile_via_ctypes("/opt/axon/libaxon_pjrt.so"))
        import concourse.bass_utils as bu
        if not getattr(bu.upload_artifacts, "_stubbed", False):
            def _noop_upload(tmpdir):
                return tmpdir
            _noop_upload._stubbed = True
            bu.upload_artifacts = _noop_upload
        return True
    except Exception:
        return False


# ---------------------------------------------------------------------------
# entry point
# ---------------------------------------------------------------------------
def kernel(codebook, starts, ends, max_i):
    global last_exec_time_ns, _last_run

    codebook = np.asarray(codebook)
    assert codebook.shape == (N, D), codebook.shape
    M = min(N, int(max_i) + 1)

    in_maps, corr_sig, act_sig = _prepare_inputs(codebook, starts, ends)

    key = (corr_sig, act_sig)
    if key not in _programs:
        _programs[key] = _build_program(corr_sig, act_sig)
    nc = _programs[key]

    trace = bool(os.environ.get("KTRACE"))
    if trace:
        trace = _install_trace_hook()
    res = run_bass_kernel_spmd(
        nc, in_maps, core_ids=list(range(NCORES)), trace=trace)
    last_exec_time_ns = res.exec_time_ns
    _last_run = res

    pos_dev = np.empty(N, dtype=np.float64)
    neg_dev = np.empty(N, dtype=np.float64)
    for c in range(NCORES):
        s = res.results[c]["sums"].astype(np.float64)  # (128, 3*T)
        s = s.reshape(128, T, 3).transpose(1, 0, 2)    # (T, 128, 3)
        off = c * ROWS_PER_CORE
        pos_dev[off:off + ROWS_PER_CORE] = s[..., 0].reshape(-1)
        neg_dev[off:off + ROWS_PER_CORE] = \
            (s[..., 1] - s[..., 2]).reshape(-1) / SC4

    return np.asarray(_host_finalize(pos_dev, neg_dev, starts, ends, M))
